# revision 37
# baseline (speedup 1.0000x reference)
"""Trainium2 Bass kernel for 4-head spatial attention + BatchNorm (dense_transformer).

Reference computation (per batch b, with n = 64*64 = 4096 spatial positions):
  qkv = W_qkv @ x            (1x1 conv == channel matmul)
  scores = (q*scale)^T k per head (head_dim 32), softmax over keys
  out = attn @ v^T ; y = W_out @ out + b_out ; BatchNorm2d over (batch, spatial)

Sharding: 8 cores = (batch b = core//2) x (n-half = core%2). Each core computes
its 2048 query positions against all 4096 keys for all 4 heads, producing the
full 256-channel output for its (b, n-half) shard. BatchNorm statistics are
all-reduced (2KB) across all 8 cores. b_out cancels inside BatchNorm and is
skipped.

Performance structure (per core, steady state):
 - scores computed TRANSPOSED: scT[m, n] = sum_d k[d,m] q[d,n] so softmax's
   reduction axis (m=keys) is the PE contraction axis downstream.
 - Per (j, mt): 4 score matmuls issued back-to-back into 4 disjoint PE row
   groups (concurrent), then for the previous mt: 4 av matmuls into 4 disjoint
   col groups (concurrent) followed by 4 den matmuls (concurrent). PE wall
   per mt ~= 3 x 512 cycles instead of ~8 serialized matmuls.
 - softmax exp is the kernel wall: 33.5M elements/core. Split across BOTH
   ScalarE (exact LUT exp, bf16 out) and VectorE (Schraudolph bit-trick:
   int16 output of x*a+b IS the bf16 pattern of 2^x, fed to the PE as a
   bitcast bf16 view). k/q/v PSUM evacuations ride ScalarE (activation Copy)
   to keep VectorE exp-dedicated.
 - den (softmax denominators) via ones-matmul rides the same rhs stream.
 - BatchNorm stats: free via accum_out on the y evacuation + one DVE
   scalar_tensor_tensor (y*y with accumulate) per tile; stats all-reduced
   across cores. A tiny dummy collective at kernel start warms up the CC
   cores so the real collective's ~11us ALGO_MESH spin-up is hidden.
 - Output written as bf16 (halves the final DMA) on the idle sync/tensor
   queues; host casts back to f32.
 - PSUM: sc [128,1024] x3 bufs (6 banks) + av/den accumulators (2 banks).
"""

import os

# This device can be left wedged by a previously-crashed process; a core
# reset at NRT init recovers it. Must be set before jax/NRT first opens the
# device (all verified timing runs had it set).
os.environ.setdefault("NEURON_RT_RESET_CORES", "1")

import numpy as np
import ml_dtypes

import concourse.bass as bass
import concourse.tile as tile
from concourse import bacc, mybir
from concourse.bass_utils import run_bass_kernel_spmd

BF16 = mybir.dt.bfloat16
F32 = mybir.dt.float32
I16 = mybir.dt.int16
AF = mybir.ActivationFunctionType
ALU = mybir.AluOpType

B, C, HW, N = 4, 256, 64, 4096
HEADS, DH, HID = 4, 32, 128
NLOC = N // 2          # 2048 query positions per core
NJ = NLOC // 512       # 4 n-chunks of 512
MT = N // 128          # 32 key tiles of 128
SCALE = float(DH) ** -0.5
EPS = 1e-5
NCORES = 8

# Schraudolph exp for the VectorE path: bf16's bit pattern for 2^x is
# ~ 128*(x + 127 - c) in int16 space; int16 output via the float->int
# conversion gives a contiguous bf16 rhs for the PE.
L2E = 1.4426950408889634
EXP_ALPHA = 128.0 * SCALE * L2E
EXP_C = 0.043750
EXP_BETA = 128.0 * (127.0 - EXP_C)

# mt values whose dp==1 exp tile goes to ScalarE instead of VectorE
# (balances ScalarE ~140 tiles + evacuations vs VectorE ~116 tiles + its
# other PSUM work).
DVE_SKIP_MTS = (8, 16, 24)


def build_nc(dve_skip=DVE_SKIP_MTS, score_grid=False, allgather=True):
    nc = bacc.Bacc("TRN2", target_bir_lowering=False)

    xq_d = nc.declare_dram_parameter("xq", [C, NLOC], BF16, isOutput=False)
    xkv_d = nc.declare_dram_parameter("xkv", [C, N], BF16, isOutput=False)
    wqT_d = nc.declare_dram_parameter("wqT", [C, HID], BF16, isOutput=False)
    wkT_d = nc.declare_dram_parameter("wkT", [C, HID], BF16, isOutput=False)
    wvT_d = nc.declare_dram_parameter("wvT", [C, HID], BF16, isOutput=False)
    woT_d = nc.declare_dram_parameter("woT", [HID, C], BF16, isOutput=False)
    gb_d = nc.declare_dram_parameter("gb", [128, 4], F32, isOutput=False)
    out_d = nc.declare_dram_parameter("out", [C, NLOC], BF16, isOutput=True)

    with tile.TileContext(nc) as tc:
        with (
            tc.tile_pool(name="consts", bufs=1) as consts,
            tc.tile_pool(name="acts", bufs=1) as acts,
            tc.tile_pool(name="expp", bufs=8) as expp,
            tc.tile_pool(name="normp", bufs=2) as normp,
            tc.tile_pool(name="outp", bufs=4) as outp,
            tc.tile_pool(name="dram", bufs=1, space="DRAM") as dram,
        ):
            # ---- persistent SBUF tensors ----
            wq_sb = consts.tile([128, 2 * HID], BF16)   # [c-chunk(2) x 128]
            wk_sb = consts.tile([128, 2 * HID], BF16)
            wv_sb = consts.tile([128, 2 * HID], BF16)
            wo_sb = consts.tile([128, C], BF16)
            gb_sb = consts.tile([128, 4], F32)
            ones_sb = consts.tile([128, 32], BF16)
            eps_sb = consts.tile([128, 1], F32)
            zrow_sb = consts.tile([128, 512], BF16)
            onerow_sb = consts.tile([1, 512], BF16)
            dummy_sb = consts.tile([128, 1], F32)
            stats_parts = consts.tile([128, 16], F32)   # [sum(ct,j) | sq(ct,j)]

            xq_sb = acts.tile([128, 2 * NLOC], BF16)    # col = cc*NLOC + n
            xkv_sb = acts.tile([128, 2 * N], BF16)      # col = cc*N + m
            q_sb = acts.tile([128, NLOC], BF16)         # part = h*32+d
            k_sb = acts.tile([128, N], BF16)            # part = h*32+d
            vT_sb = acts.tile([128, N], BF16)           # col = mt*128 + h*32 + d
            avn_sb = acts.tile([128, NLOC], BF16)       # normalized attn@v
            y_sb = acts.tile([128, 2 * NLOC], F32)      # col = ct*NLOC + n
            sq_sb = acts.tile([128, 1024], F32)         # DVE y*y scratch
            stats_sb = acts.tile([128, 4], F32)
            statsr_sb = acts.tile([128, 4], F32)

            dummy_in = dram.tile([128, 1], F32)
            dummy_out = dram.tile([128, 1], F32)
            stats_in = dram.tile([128, 4], F32)
            if allgather:
                stats_out = dram.tile([NCORES * 128, 4], F32)
            else:
                stats_out = dram.tile([128, 4], F32)

            # ---- warm-up collective: absorbs the CC cores' ~11us
            # ALGO_MESH spin-up and the cross-core launch stagger while
            # the main engines are loading inputs / projecting.
            nc.vector.memset(dummy_sb[:], 0.0)
            nc.sync.dma_start(out=dummy_in[:], in_=dummy_sb[:])
            nc.gpsimd.collective_compute(
                "AllReduce",
                ALU.add,
                replica_groups=[list(range(NCORES))],
                ins=[dummy_in.opt()],
                outs=[dummy_out.opt()],
            )

            # ---- load inputs: few large DMAs, spread across engine queues
            # (each DMA trigger costs ~700ns of queue time, so parallelize)
            # Arrival-ordered input load: the whole input set (~5MB) is
            # HBM-bandwidth-bound (~11us), so the pieces the pipeline needs
            # FIRST (wk/wq, xkv/xq cols 0-511) go first on each queue; the
            # bulk streams behind while the first key tiles compute.
            # All input triggers ride the sync/gpsimd queues: DMA triggers
            # cost ~1-2us of queue time each and must NOT sit ahead of the
            # k/q evacuation copies and exps on the scalar queue.
            for cc in range(2):
                nc.sync.dma_start(
                    out=wk_sb[:, cc * HID:(cc + 1) * HID],
                    in_=wkT_d[cc * 128:(cc + 1) * 128, :],
                )
                nc.gpsimd.dma_start(
                    out=wq_sb[:, cc * HID:(cc + 1) * HID],
                    in_=wqT_d[cc * 128:(cc + 1) * 128, :],
                )
            nc.sync.dma_start(
                out=xkv_sb[:, 0:512], in_=xkv_d[0:128, 0:512]
            )
            nc.gpsimd.dma_start(
                out=xkv_sb[:, N:N + 512], in_=xkv_d[128:256, 0:512]
            )
            nc.sync.dma_start(out=xq_sb[:, 0:512], in_=xq_d[0:128, 0:512])
            nc.gpsimd.dma_start(
                out=xq_sb[:, NLOC:NLOC + 512], in_=xq_d[128:256, 0:512]
            )
            for cc in range(2):
                nc.gpsimd.dma_start(
                    out=wv_sb[:, cc * HID:(cc + 1) * HID],
                    in_=wvT_d[cc * 128:(cc + 1) * 128, :],
                )
            for lo, hi in [(512, 2048), (2048, 4096)]:
                nc.sync.dma_start(
                    out=xkv_sb[:, lo:hi], in_=xkv_d[0:128, lo:hi]
                )
                nc.gpsimd.dma_start(
                    out=xkv_sb[:, N + lo:N + hi], in_=xkv_d[128:256, lo:hi]
                )
            nc.sync.dma_start(out=xq_sb[:, 512:NLOC], in_=xq_d[0:128, 512:NLOC])
            nc.gpsimd.dma_start(
                out=xq_sb[:, NLOC + 512:2 * NLOC], in_=xq_d[128:256, 512:NLOC]
            )
            nc.sync.dma_start(out=wo_sb[:], in_=woT_d[:])
            nc.gpsimd.dma_start(out=gb_sb[:], in_=gb_d[:])
            nc.vector.memset(ones_sb[:], 1.0)
            nc.vector.memset(eps_sb[:], EPS)
            nc.vector.memset(zrow_sb[:], 0.0)
            nc.vector.memset(onerow_sb[:], 1.0)

            with (
                tc.tile_pool(name="scps", bufs=3, space="PSUM") as scps,
                tc.tile_pool(name="avps", bufs=1, space="PSUM") as avps,
            ):
                # ---- projections, using the attention PSUM pools so the
                # whole kernel is one pipelined region. Emission order is
                # interleaved with the DMA chunk arrivals. PSUM->SBUF
                # evacuations go to ScalarE (activation Copy) so VectorE
                # stays exp-dedicated.
                def khalf(jj):  # 512 keys per piece
                    kt = scps.tile([128, 1024], F32, tag="sc")
                    for cc in range(2):
                        nc.tensor.matmul(
                            kt[:, 0:512],
                            lhsT=wk_sb[:, cc * HID:(cc + 1) * HID],
                            rhs=xkv_sb[:, cc * N + jj * 512: cc * N + jj * 512 + 512],
                            start=(cc == 0), stop=(cc == 1),
                        )
                    nc.scalar.copy(k_sb[:, jj * 512:(jj + 1) * 512], kt[:, 0:512])

                def qhalf(jj):  # 512 query positions per piece
                    qt = scps.tile([128, 1024], F32, tag="sc")
                    for cc in range(2):
                        nc.tensor.matmul(
                            qt[:, 0:512],
                            lhsT=wq_sb[:, cc * HID:(cc + 1) * HID],
                            rhs=xq_sb[:, cc * NLOC + jj * 512: cc * NLOC + jj * 512 + 512],
                            start=(cc == 0), stop=(cc == 1),
                        )
                    nc.scalar.copy(q_sb[:, jj * 512:(jj + 1) * 512], qt[:, 0:512])

                def vgroup(g):  # 4 mt-tiles per group, via an sc-pool slot
                    # (half used); interleaved into j0's mt loop so the
                    # 64-matmul v projection hides under the attention
                    # pipeline instead of serializing the startup.
                    vt = scps.tile([128, 1024], F32, tag="sc")
                    for mi in range(4):
                        mt = g * 4 + mi
                        for cc in range(2):
                            nc.tensor.matmul(
                                vt[:, mi * 128: mi * 128 + 128],
                                lhsT=xkv_sb[:, cc * N + mt * 128: cc * N + mt * 128 + 128],
                                rhs=wv_sb[:, cc * HID:(cc + 1) * HID],
                                start=(cc == 0), stop=(cc == 1),
                            )
                    nc.scalar.copy(vT_sb[:, g * 512:(g + 1) * 512], vt[:, 0:512])

                # Minimal prologue: scores(j0, mt0..3) only need keys 0-511
                # and queries 0-511, so the exp engines start ~13us earlier
                # than with all projections up front. The remaining k/q
                # pieces and the v projection spread through j0's mt loop
                # (vgroup(g) isn't consumed until iteration 4g+2), keeping
                # PSUM pool pressure at <=3 allocations per iteration.
                khalf(0)
                qhalf(0)
                J0_EXTRAS = {
                    0: lambda: khalf(1),
                    1: lambda: vgroup(0),
                    2: lambda: khalf(2),
                    3: lambda: khalf(3),
                    4: lambda: vgroup(1),
                    5: lambda: khalf(4),
                    6: lambda: khalf(5),
                    7: lambda: khalf(6),
                    8: lambda: vgroup(2),
                    9: lambda: khalf(7),
                    10: lambda: qhalf(1),
                    11: lambda: qhalf(2),
                    12: lambda: vgroup(3),
                    13: lambda: qhalf(3),
                    16: lambda: vgroup(4),
                    20: lambda: vgroup(5),
                    24: lambda: vgroup(6),
                    27: lambda: vgroup(7),
                }

                # ---- attention ----
                def emit_quad(st):
                    # av as 4 concurrent col-group matmuls (M=32 each), den
                    # as 4 concurrent M=1 ones-matmuls whose single output
                    # row sits at partition 32h of its col group — the den
                    # PSUM write traffic is 1/32nd of a broadcast quad; the
                    # per-head broadcast happens once per j via a tiny f32
                    # block-ones matmul instead.
                    ex_list, av_, den_, mt_, j_, last_ = st
                    for h in range(4):
                        dp, hh = h // 2, h % 2
                        ex, is_dve = ex_list[dp]
                        if is_dve:
                            rhs = ex[:].bitcast(BF16)[:, hh * 512:(hh + 1) * 512]
                        else:
                            rhs = ex[:, hh * 512:(hh + 1) * 512]
                        nc.tensor.matmul(
                            av_[32 * h:32 * h + 32, :],
                            lhsT=vT_sb[:, mt_ * 128 + 32 * h: mt_ * 128 + 32 * h + 32],
                            rhs=rhs,
                            start=False, stop=last_,
                            tile_position=(0, 32 * h),
                            skip_group_check=True,
                        )
                    for h in range(4):
                        dp, hh = h // 2, h % 2
                        ex, is_dve = ex_list[dp]
                        if is_dve:
                            rhs = ex[:].bitcast(BF16)[:, hh * 512:(hh + 1) * 512]
                        else:
                            rhs = ex[:, hh * 512:(hh + 1) * 512]
                        # ones-matmul: softmax denominator broadcast
                        # across the head's 32 partitions
                        nc.tensor.matmul(
                            den_[32 * h:32 * h + 32, :],
                            lhsT=ones_sb[:, 0:32],
                            rhs=rhs,
                            start=False, stop=last_,
                            tile_position=(0, 32 * h),
                            skip_group_check=True,
                        )

                pending = []
                for j in range(NJ):
                    av_ps = avps.tile([128, 512], F32, tag="av")
                    den_ps = avps.tile([128, 512], F32, tag="den")
                    # open whole-bank accumulation groups with K=1 MMs.
                    # den opens to all-ones so reciprocal_approx of the
                    # unused rows is well-defined; the +1 on the real rows
                    # is ~1.5e-4 of a 4096-key exp-sum.
                    nc.tensor.matmul(
                        av_ps[:], lhsT=zrow_sb[0:1, 0:128], rhs=zrow_sb[0:1, :],
                        start=True, stop=False, skip_group_check=True,
                    )
                    nc.tensor.matmul(
                        den_ps[:], lhsT=onerow_sb[0:1, 0:128], rhs=onerow_sb[0:1, :],
                        start=True, stop=False, skip_group_check=True,
                    )
                    for mt in range(MT):
                        if j == 0 and mt in J0_EXTRAS:
                            J0_EXTRAS[mt]()
                        sc_tiles = []
                        for dp in range(2):
                            sc = scps.tile([128, 1024], F32, tag="sc")
                            for hh in range(2):
                                h = dp * 2 + hh
                                if score_grid:
                                    # 4h x 4kk grid of K=32,M=32 sub-MMs:
                                    # each writes a 32-partition slice, so
                                    # concurrent tiles share the PSUM write
                                    # port instead of serializing on it.
                                    for kk in range(4):
                                        nc.tensor.matmul(
                                            sc[32 * kk:32 * kk + 32, hh * 512:(hh + 1) * 512],
                                            lhsT=k_sb[32 * h:32 * h + 32, mt * 128 + 32 * kk: mt * 128 + 32 * kk + 32],
                                            rhs=q_sb[32 * h:32 * h + 32, j * 512:(j + 1) * 512],
                                            start=True, stop=True,
                                            tile_position=(32 * h, 32 * kk),
                                            skip_group_check=True,
                                        )
                                else:
                                    nc.tensor.matmul(
                                        sc[:, hh * 512:(hh + 1) * 512],
                                        lhsT=k_sb[32 * h:32 * h + 32, mt * 128:(mt + 1) * 128],
                                        rhs=q_sb[32 * h:32 * h + 32, j * 512:(j + 1) * 512],
                                        start=True, stop=True,
                                        tile_position=(32 * h, 0),
                                    )
                            sc_tiles.append(sc)
                        ex_list = []
                        for dp in range(2):
                            sc = sc_tiles[dp]
                            use_dve = dp == 1 and mt not in dve_skip
                            if use_dve:
                                ex = expp.tile([128, 1024], I16, tag="exi")
                                nc.vector.tensor_scalar(
                                    out=ex[:], in0=sc[:],
                                    scalar1=EXP_ALPHA, scalar2=EXP_BETA,
                                    op0=ALU.mult, op1=ALU.add,
                                )
                            else:
                                ex = expp.tile([128, 1024], BF16, tag="ex")
                                nc.scalar.activation(
                                    ex[:], sc[:], AF.Exp, scale=SCALE
                                )
                            ex_list.append((ex, use_dve))
                        # depth-2 pipeline: emit av/den for mt-2, so those
                        # matmuls consume exp tiles finished long ago and
                        # the PE never micro-idles waiting on the exp
                        # engines (micro-idles oscillate the HAM clock
                        # gate and halve the effective PE clock).
                        if len(pending) >= 2:
                            emit_quad(pending.pop(0))
                        pending.append(
                            (ex_list, av_ps, den_ps, mt, j, mt == MT - 1)
                        )
                    while pending:
                        emit_quad(pending.pop(0))

                    # normalize: avn = av * (1/den)
                    rden_sb = normp.tile([128, 512], F32, tag="rden")
                    nc.vector.reciprocal_approx_fast(out=rden_sb[:], in_=den_ps[:])
                    nc.vector.tensor_mul(
                        avn_sb[:, j * 512:(j + 1) * 512], av_ps[:], rden_sb[:]
                    )

                    # y projection for this j (sc-pool slot, halves = ct)
                    y_ps = scps.tile([128, 1024], F32, tag="sc")
                    for ct in range(2):
                        nc.tensor.matmul(
                            y_ps[:, ct * 512: ct * 512 + 512],
                            lhsT=wo_sb[:, ct * 128:(ct + 1) * 128],
                            rhs=avn_sb[:, j * 512:(j + 1) * 512],
                            start=True, stop=True,
                        )
                    for ct in range(2):
                        ysl = y_sb[:, ct * NLOC + j * 512: ct * NLOC + j * 512 + 512]
                        # evacuate + batch-mean partial for free via accum_out
                        # (ct=1 evac on ScalarE so the two y evacuations run
                        # concurrently — shortens the j3 serial tail)
                        if ct == 0:
                            nc.vector.tensor_scalar(
                                out=ysl,
                                in0=y_ps[:, ct * 512: ct * 512 + 512],
                                scalar1=1.0, scalar2=0.0,
                                op0=ALU.mult, op1=ALU.add,
                                accum_out=stats_parts[:, ct * 4 + j: ct * 4 + j + 1],
                            )
                        else:
                            nc.scalar.activation(
                                ysl, y_ps[:, ct * 512: ct * 512 + 512],
                                AF.Copy,
                                accum_out=stats_parts[:, ct * 4 + j: ct * 4 + j + 1],
                            )
                        # sum(y^2) partial: one DVE pass (y*y + accumulate)
                        nc.vector.scalar_tensor_tensor(
                            out=sq_sb[:, ct * 512: ct * 512 + 512],
                            in0=ysl, scalar=1.0, in1=ysl,
                            op0=ALU.mult, op1=ALU.mult,
                            accum_out=stats_parts[:, 8 + ct * 4 + j: 9 + ct * 4 + j],
                        )

            # ---- BatchNorm stats (b_out cancels in BN); stats are
            # pre-scaled by 1/N before the collective so the post-collective
            # critical path is as short as possible. ----
            inv_n = 1.0 / float(B * N)
            nc.vector.tensor_reduce(
                stats_sb[:, 0:2],
                stats_parts[:, 0:8].rearrange("p (c j) -> p c j", c=2),
                axis=mybir.AxisListType.X, op=ALU.add,
            )
            nc.vector.tensor_reduce(
                stats_sb[:, 2:4],
                stats_parts[:, 8:16].rearrange("p (c j) -> p c j", c=2),
                axis=mybir.AxisListType.X, op=ALU.add,
            )
            nc.vector.tensor_scalar_mul(stats_sb[:], stats_sb[:], inv_n)
            # stats DMAs ride the sync queue: the gpsimd queue's post-DMA
            # DRAIN is ~3.3us and would sit on the post-collective critical
            # path.
            nc.sync.dma_start(out=stats_in[:], in_=stats_sb[:])
            if allgather:
                # AllGather + local reduce: fewer post-wait mesh phases on
                # the critical path than AllReduce.
                nc.gpsimd.collective_compute(
                    "AllGather",
                    ALU.bypass,
                    replica_groups=[list(range(NCORES))],
                    ins=[stats_in.opt()],
                    outs=[stats_out.opt()],
                )
                statsg_sb = acts.tile([128, 4 * NCORES], F32)
                # dram layout: [(core, part), 4] -> sbuf [part, (core, 4)]
                nc.sync.dma_start(
                    out=statsg_sb[:].rearrange("p (k c) -> p k c", k=NCORES),
                    in_=stats_out[:].rearrange("(k p) c -> p k c", k=NCORES),
                )
                nc.vector.tensor_reduce(
                    statsr_sb[:],
                    statsg_sb[:].rearrange("p (k c) -> p c k", k=NCORES),
                    axis=mybir.AxisListType.X, op=ALU.add,
                )
            else:
                nc.gpsimd.collective_compute(
                    "AllReduce",
                    ALU.add,
                    replica_groups=[list(range(NCORES))],
                    ins=[stats_in.opt()],
                    outs=[stats_out.opt()],
                )
                nc.sync.dma_start(out=statsr_sb[:], in_=stats_out[:])

            tmp_sb = consts.tile([128, 2], F32)
            var_sb = consts.tile([128, 2], F32)
            std_sb = consts.tile([128, 2], F32)
            rstd_sb = consts.tile([128, 2], F32)
            scal_sb = consts.tile([128, 2], F32)
            bias_sb = consts.tile([128, 2], F32)
            mean = statsr_sb[:, 0:2]
            nc.vector.tensor_mul(tmp_sb[:], mean, mean)
            nc.vector.tensor_sub(var_sb[:], statsr_sb[:, 2:4], tmp_sb[:])
            nc.scalar.activation(
                std_sb[:], var_sb[:], AF.Sqrt, bias=eps_sb[:, 0:1]
            )
            nc.vector.reciprocal(rstd_sb[:], std_sb[:])
            nc.vector.tensor_mul(scal_sb[:], gb_sb[:, 0:2], rstd_sb[:])
            nc.vector.tensor_mul(tmp_sb[:], mean, scal_sb[:])
            nc.vector.tensor_sub(bias_sb[:], gb_sb[:, 2:4], tmp_sb[:])

            # ---- apply: out = y*scale + bias as bf16 (halves the store),
            # ch=0 chunks on DVE, ch=1 chunks on ScalarE (Identity with
            # per-partition scale/bias) so the two halves run concurrently.
            for ct in range(2):
                for ch in range(2):
                    yo = outp.tile([128, NLOC // 2], BF16, tag="yo")
                    ysl = y_sb[:, ct * NLOC + ch * 1024: ct * NLOC + ch * 1024 + 1024]
                    if ch == 0:
                        nc.vector.tensor_scalar(
                            out=yo[:], in0=ysl,
                            scalar1=scal_sb[:, ct:ct + 1],
                            scalar2=bias_sb[:, ct:ct + 1],
                            op0=ALU.mult, op1=ALU.add,
                        )
                    else:
                        nc.scalar.activation(
                            yo[:], ysl, AF.Identity,
                            scale=scal_sb[:, ct:ct + 1],
                            bias=bias_sb[:, ct:ct + 1],
                        )
                    dma_eng = nc.sync if ch == 0 else nc.scalar
                    dma_eng.dma_start(
                        out=out_d[ct * 128:(ct + 1) * 128, ch * 1024:(ch + 1) * 1024],
                        in_=yo[:],
                    )

    nc.compile()
    return nc


_NC_CACHE = {}


def _get_nc():
    if "nc" not in _NC_CACHE:
        _NC_CACHE["nc"] = build_nc()
    return _NC_CACHE["nc"]


def kernel(x, W_qkv, W_out, b_out, gamma, beta):
    bf16 = ml_dtypes.bfloat16
    x = np.asarray(x, np.float32)
    W_qkv = np.asarray(W_qkv, np.float32)
    W_out = np.asarray(W_out, np.float32)
    gamma = np.asarray(gamma, np.float32)
    beta = np.asarray(beta, np.float32)

    xf = x.reshape(B, C, N)
    wqT = np.ascontiguousarray(W_qkv[0:HID, :].T).astype(bf16)
    wkT = np.ascontiguousarray(W_qkv[HID:2 * HID, :].T).astype(bf16)
    wvT = np.ascontiguousarray(W_qkv[2 * HID:3 * HID, :].T).astype(bf16)
    woT = np.ascontiguousarray(W_out.T).astype(bf16)
    gb = np.stack(
        [gamma[0:128], gamma[128:256], beta[0:128], beta[128:256]], axis=1
    ).astype(np.float32)
    gb = np.ascontiguousarray(gb)

    in_maps = []
    for core in range(NCORES):
        b, half = core // 2, core % 2
        xb = np.ascontiguousarray(xf[b]).astype(bf16)
        xq = np.ascontiguousarray(xb[:, half * NLOC:(half + 1) * NLOC])
        in_maps.append({
            "xq": xq, "xkv": xb,
            "wqT": wqT, "wkT": wkT, "wvT": wvT, "woT": woT, "gb": gb,
        })

    nc = _get_nc()
    _NC_CACHE["last_in_maps"] = in_maps
    res = run_bass_kernel_spmd(nc, in_maps, core_ids=list(range(NCORES)))

    out = np.empty((B, C, N), np.float32)
    for core in range(NCORES):
        b, half = core // 2, core % 2
        out[b][:, half * NLOC:(half + 1) * NLOC] = np.asarray(
            res.results[core]["out"], np.float32
        )
    return out.reshape(B, C, HW, HW)
